# revision 29
# baseline (speedup 1.0000x reference)
"""Distributed brute-force KNN (retrieval) kernel for one TRN2 chip (8 NeuronCores).

Problem: queries [256,128] f32, candidates [500000,128] f32, identifiers [500000] i32,
k=100. Output: (values [256,100] f32 desc-sorted, ids [256,100] i32).

Device strategy (per core, candidates sharded 8 x 62500):
  - bf16 matmul: Q half (128 queries) stationary, candidate supertiles of 2048
    streamed -> PSUM [128, 2048] f32 (4 banks, 4 x 512-col matmuls).
  - Reduction to per-column running max ("claims") via two routes that keep
    ScalarE and VectorE both busy:
      route A: ScalarE copies PSUM f32 -> SBUF bf16 stage (1x), VectorE folds
               stage into a bf16 ping/pong accumulator at 2x.
      route V: VectorE tensor_max directly from PSUM into the accumulator (1x).
  - End: fold acc 2048 -> 1024, DMA claims [256, 1024] bf16 out.
  - Claims are window maxes: col j of core c covers candidates
    {c*62500 + st*2048 + j + d*1024 : st in 0..30, d in 0..1} (62 members).

Host: rescores claimed windows exactly in f64 until every window that could
still hide a top-k entry (claim >= thr - margin) has been scanned. Exactness
never depends on device numerics.
"""
import numpy as np
import ml_dtypes

B = 256          # queries
N = 500000       # candidates
D = 128          # dim
NCORES = 8
NSH = N // NCORES            # 62500 real candidates per core
STW = 2048                   # candidates per supertile (4 PSUM banks)
NST = 31                     # supertiles per core
NP = NST * STW               # 63488 padded candidates per core
CW = STW // 2                # claim columns per (core, query)
NWIN = NCORES * CW           # windows per query
WMEM = NST * 2               # members per window (62)

_CACHE = {}


def build(loops=1, pattern="aaav", staggered=True, warmup=32):
    """Build + compile the per-core Bass program.

    pattern: route per visit (visit = (supertile, half)), cycled.
             'a' = ScalarE evacuate + VectorE bf16 fold,
             'v' = VectorE direct from PSUM,
             'g' = ScalarE evacuate + GpSimd bf16 fold.
    warmup: one-time pre-loop burst of back-to-back matmuls (>=6.8us cold) so
            the PE HAM un-throttles to 2.4GHz; mid-loop gaps stay <3.4us so it
            never re-throttles.
    """
    import concourse.bass as bass
    import concourse.tile as tile
    from concourse import bacc, mybir

    bf16 = mybir.dt.bfloat16
    f32 = mybir.dt.float32
    Copy = mybir.ActivationFunctionType.Copy

    nc = bacc.Bacc("TRN2", debug=False)
    qt = nc.dram_tensor("qt", [D, B], bf16, kind="ExternalInput").ap()
    ct = nc.dram_tensor("ct", [D, NP], bf16, kind="ExternalInput").ap()
    cl = nc.dram_tensor("cl", [B, CW], bf16, kind="ExternalOutput").ap()
    dbg = nc.dram_tensor("dbg", [128, 8], f32, kind="ExternalOutput").ap()
    use_g = "g" in pattern
    NMM = STW // 512

    with tile.TileContext(nc) as tc:
        with (
            tc.tile_pool(name="qpool", bufs=1) as qpool,
            tc.tile_pool(name="cpool", bufs=3) as cpool,
            tc.tile_pool(name="spool", bufs=6) as spool,
            tc.tile_pool(name="apool", bufs=1) as apool,
            tc.tile_pool(name="psum", bufs=2, space="PSUM") as pp,
        ):
            qtile = qpool.tile([D, B], bf16)
            nc.sync.dma_start(qtile[:], qt[:])
            acc = [
                [
                    apool.tile([128, STW], bf16, tag=f"acc{h}{p}", name=f"acc{h}{p}")
                    for p in range(2)
                ]
                for h in range(2)
            ]
            gacc = [
                [
                    apool.tile([128, STW], bf16, tag=f"gac{h}{p}", name=f"gac{h}{p}")
                    for p in range(2)
                ]
                for h in range(2)
            ] if use_g else None
            claims = [
                apool.tile([128, CW], bf16, tag=f"clm{h}", name=f"clm{h}")
                for h in range(2)
            ]
            if warmup:
                dread = apool.tile([128, 8], f32, tag="dread", name="dread")
                wu = pp.tile([128, STW], f32, tag="ps", name="wu")
                for i in range(warmup):
                    nc.tensor.matmul(
                        wu[:, bass.ds((i % 4) * 512, 256)],
                        lhsT=qtile[:, bass.ds(0, 128)],
                        rhs=qtile[:, bass.ds(0, 256)],
                        start=True,
                        stop=True,
                    )
                nc.vector.tensor_copy(dread[:], wu[:, bass.ds(0, 8)])
                nc.sync.dma_start(dbg[:], dread[:])

            def body(_iv=None):
                cur = [None, None]   # ping/pong pointer per half (DVE acc)
                gcur = [None, None]  # ping/pong pointer per half (GpSimd acc)
                vis = 0
                for st in range(NST):
                    cchunk = cpool.tile([D, STW], bf16, tag="ct", name="cchunk")
                    nc.sync.dma_start(
                        cchunk[:], ct[:, bass.ds(st * STW, STW)]
                    )
                    for h in range(2):
                        ps = pp.tile([128, STW], f32, tag="ps", name="ps")
                        for j in range(NMM):
                            nc.tensor.matmul(
                                ps[:, bass.ds(j * 512, 512)],
                                lhsT=qtile[:, bass.ds(h * 128, 128)],
                                rhs=cchunk[:, bass.ds(j * 512, 512)],
                                start=True,
                                stop=True,
                            )
                        route = pattern[vis % len(pattern)]
                        vis += 1
                        if route == "x":
                            # PE-rate probe: trivial consumer, claims garbage
                            nc.vector.tensor_copy(
                                acc[h][0][:, bass.ds(0, 8)], ps[:, bass.ds(0, 8)]
                            )
                            cur[h] = 0
                        elif route == "v":
                            if cur[h] is None:
                                nc.vector.tensor_copy(acc[h][0][:], ps[:])
                                cur[h] = 0
                            else:
                                p = cur[h]
                                nc.vector.tensor_max(
                                    acc[h][1 - p][:], ps[:], acc[h][p][:]
                                )
                                cur[h] = 1 - p
                        elif route == "a":
                            if cur[h] is None:
                                nc.scalar.activation(acc[h][0][:], ps[:], Copy)
                                cur[h] = 0
                            else:
                                stg = spool.tile(
                                    [128, STW], bf16, tag="stg", name="stg"
                                )
                                nc.scalar.activation(stg[:], ps[:], Copy)
                                p = cur[h]
                                nc.vector.tensor_max(
                                    acc[h][1 - p][:], stg[:], acc[h][p][:]
                                )
                                cur[h] = 1 - p
                        else:  # 'g'
                            if gcur[h] is None:
                                nc.scalar.activation(gacc[h][0][:], ps[:], Copy)
                                gcur[h] = 0
                            else:
                                stg = spool.tile(
                                    [128, STW], bf16, tag="stg", name="stg"
                                )
                                nc.scalar.activation(stg[:], ps[:], Copy)
                                p = gcur[h]
                                nc.gpsimd.tensor_max(
                                    gacc[h][1 - p][:], stg[:], gacc[h][p][:]
                                )
                                gcur[h] = 1 - p
                for h in range(2):
                    a = acc[h][cur[h]]
                    if gcur[h] is not None:
                        g = gacc[h][gcur[h]]
                        m = acc[h][1 - cur[h]]
                        nc.vector.tensor_max(m[:], a[:], g[:])
                        a = m
                    nc.vector.tensor_max(
                        claims[h][:], a[:, bass.ds(0, CW)], a[:, bass.ds(CW, CW)]
                    )
                    nc.sync.dma_start(cl[bass.ds(h * 128, 128), :], claims[h][:])

            if loops == 1:
                body()
            else:
                with tc.For_i(
                    0,
                    loops,
                    1,
                    hint_engines=(mybir.EngineType.PE,),
                    staggered_reset=staggered,
                ) as iv:
                    body(iv)
    nc.compile()
    return nc


def _get_nc():
    if "nc" not in _CACHE:
        _CACHE["nc"] = build()
    return _CACHE["nc"]


def make_in_maps(queries, candidates):
    qt = np.ascontiguousarray(queries.T).astype(ml_dtypes.bfloat16)
    cb = candidates.astype(ml_dtypes.bfloat16)
    in_maps = []
    for c in range(NCORES):
        ct = np.zeros((D, NP), dtype=ml_dtypes.bfloat16)
        ct[:, :NSH] = cb[c * NSH : (c + 1) * NSH].T
        in_maps.append({"qt": qt, "ct": ct})
    return in_maps


def _device_claims(queries, candidates):
    """Run the 8-core SPMD kernel; return claims [B, NCORES*CW] f32."""
    from concourse.bass_utils import run_bass_kernel_spmd

    nc = _get_nc()
    in_maps = make_in_maps(queries, candidates)
    res = None
    for attempt in range(3):
        try:
            res = run_bass_kernel_spmd(
                nc, in_maps, core_ids=list(range(NCORES))
            ).results
            break
        except Exception:
            if attempt == 2:
                raise
            import time as _time

            _time.sleep(2.0)
    assert res is not None
    cls = np.stack([r["cl"] for r in res]).astype(np.float32)  # [8, B, CW]
    return cls.transpose(1, 0, 2).reshape(B, NWIN)  # window w = c*CW + j


# ---- host-side exact validation ------------------------------------------

# member table: window col j -> local candidate offsets (shared by all cores)
_ST = np.arange(NST) * STW
_MEMLOC = (
    _ST[None, :, None]
    + np.arange(2)[None, None, :] * CW
    + np.arange(CW)[:, None, None]
).reshape(CW, WMEM)                                   # [CW, 62]
_MVALID = _MEMLOC < NSH                               # [CW, 62]


def _rescan(qidx, widx, candidates, q64):
    """Exact f64 scores for windows widx of queries qidx (flat pair lists).

    Returns (scores [P, WMEM] f64 with -inf at invalid, gidx [P, WMEM] int64
    with -1 at invalid, wmax [P] f64).
    """
    c = widx // CW
    j = widx % CW
    loc = _MEMLOC[j]                                  # [P, 62]
    valid = _MVALID[j]
    g = loc + (c * NSH)[:, None]
    gs = np.where(valid, g, 0)
    # [P, 62, D] gather -> dot with per-pair query
    sv = np.einsum(
        "pmd,pd->pm", candidates[gs].astype(np.float64), q64[qidx], optimize=True
    )
    sv = np.where(valid, sv, -np.inf)
    return sv, np.where(valid, g, -1), sv.max(axis=1)


def kernel(queries, candidates, identifiers, k):
    queries = np.asarray(queries, dtype=np.float32)
    candidates = np.asarray(candidates, dtype=np.float32)
    identifiers = np.asarray(identifiers)
    kk = int(k)

    claims = _device_claims(queries, candidates)      # [B, NWIN] f32
    q64 = queries.astype(np.float64)
    sigma = np.linalg.norm(queries, axis=1)

    scanned = np.zeros((B, NWIN), dtype=bool)
    pool_v = [None] * B
    pool_g = [None] * B
    delta = np.zeros(B)

    def scan(qidx, widx):
        """Rescan pairs (qidx[i], widx[i]); update pools and deltas."""
        if qidx.size == 0:
            return
        CH = 4096
        for lo in range(0, qidx.size, CH):
            qi = qidx[lo : lo + CH]
            wi = widx[lo : lo + CH]
            sv, gi, wmax = _rescan(qi, wi, candidates, q64)
            d = np.abs(claims[qi, wi] - wmax)
            np.maximum.at(delta, qi, d)
            for t in range(qi.size):
                q = qi[t]
                if pool_v[q] is None:
                    pool_v[q] = [sv[t]]
                    pool_g[q] = [gi[t]]
                else:
                    pool_v[q].append(sv[t])
                    pool_g[q].append(gi[t])
            scanned[qi, wi] = True

    # round 0: top-L windows per query by claim
    L0 = max(32, (kk + WMEM - 1) // WMEM + 16)
    part = np.argpartition(-claims, L0, axis=1)[:, :L0]
    qidx0 = np.repeat(np.arange(B), L0)
    scan(qidx0, part.ravel())

    for _ in range(8):
        thr = np.empty(B)
        for q in range(B):
            vs = np.concatenate(pool_v[q])
            vs = vs[np.isfinite(vs)]
            assert vs.size >= kk
            thr[q] = -np.partition(-vs, kk - 1)[kk - 1]
        margin = np.maximum(1.0, 4.0 * delta) + 1e-3 * sigma
        need = (claims >= (thr - margin)[:, None]) & ~scanned
        if not need.any():
            break
        qi, wi = np.nonzero(need)
        scan(qi, wi)
    else:
        raise RuntimeError("window rescan did not converge")

    out_v = np.empty((B, kk), np.float32)
    out_g = np.empty((B, kk), np.int64)
    for q in range(B):
        v = np.concatenate(pool_v[q]).ravel()
        g = np.concatenate(pool_g[q]).ravel()
        keep = g >= 0
        v32 = v[keep].astype(np.float32)
        gk = g[keep]
        order = np.lexsort((gk, -v32))[:kk]
        out_v[q] = v32[order]
        out_g[q] = gk[order]

    top_ids = identifiers[out_g]
    return out_v, top_ids
